# revision 1
# baseline (speedup 1.0000x reference)
"""Trainium2 Bass kernel for nn_CausalLTXAttention (sliding-window + sink causal attention).

Sharding: 8 cores = 2 batches x 4 sequence blocks of 512 queries each.
Each core computes QKV projections for its block (+ a 511-key halo + the sink
key 0), RMSNorm + interleaved RoPE (norm weights / logit scale / 1/sqrt(dh)
folded into host-precomputed cos/sin tables), banded-causal attention with
max-free softmax (scores bounded ~7.5), and the output projection for its own
512 rows.  Outputs are disjoint -> host just concatenates (and adds bo).

Device layout notes:
  - q/k are computed in [l,e] layout (rmsnorm over free dim), roped, then
    PE-transposed into qT/kT [e,l] tiles for the attention matmuls.
  - scores are computed transposed: S^T[k,l_q] tiles; softmax denominator is
    obtained by augmenting V with a ones column in the PV matmul; the
    reciprocal row is partition-broadcast on GpSimd.
  - matmul operands are bitcast to float32r (fast PE mode, N=512 moving dim).
"""

import os
from contextlib import ExitStack

import numpy as np
import ml_dtypes

import concourse.bass as bass
import concourse.bacc as bacc
import concourse.mybir as mybir
import concourse.tile as tile
from concourse.bass_utils import run_bass_kernel_spmd
from concourse.masks import make_identity

# ---- problem constants (hardcoded per the harness contract) ----
B, L, D = 2, 2048, 2048
H, DH = 16, 64
INNER = H * DH  # 1024
WINDOW, SINK = 512, 1
EPS = 1e-6
NCORES, NBLK = 8, 4
BLK = L // NBLK  # 512 queries per core
KC = 1024  # key cols per core: [sink | halo 511 | own 512]
NLK = KC // 128  # 8 key l-tiles
NLQ = BLK // 128  # 4 own l-tiles
ND = D // 128  # 16 contraction d-tiles
NE2 = INNER // 512  # 2 e-chunks for projections
NET = INNER // 128  # 8 e-tiles
NDC = D // 512  # 4 output d-chunks
VW = H * (DH + 1)  # 1040: v tiles with a ones column per head

F32 = mybir.dt.float32
F32R = mybir.dt.float32r
BF16 = mybir.dt.bfloat16

MM_FAST = os.environ.get("MM_DT", "f32r") != "f32"


def _mm(ap):
    """Matmul operands are declared float32r at the tensor level; identity here."""
    return ap


def _build(with_bias: bool):
    nc = bacc.Bacc("TRN2", target_bir_lowering=False, debug=False)

    xT = nc.dram_tensor("xT", [D, KC], BF16, kind="ExternalInput")
    wqT = nc.dram_tensor("wqT", [D, INNER], BF16, kind="ExternalInput")
    wkT = nc.dram_tensor("wkT", [D, INNER], BF16, kind="ExternalInput")
    wvT = nc.dram_tensor("wvT", [D, INNER], BF16, kind="ExternalInput")
    woT = nc.dram_tensor("woT", [INNER, D], BF16, kind="ExternalInput")
    qtc = nc.dram_tensor("qtc", [BLK, INNER], F32, kind="ExternalInput")
    qts = nc.dram_tensor("qts", [BLK, INNER], F32, kind="ExternalInput")
    ktc = nc.dram_tensor("ktc", [KC, INNER], F32, kind="ExternalInput")
    kts = nc.dram_tensor("kts", [KC, INNER], F32, kind="ExternalInput")
    msk = nc.dram_tensor("msk", [NLK, 128, BLK], BF16, kind="ExternalInput")
    vcl = nc.dram_tensor("vcl", [NLK, 128], F32, kind="ExternalInput")
    if with_bias:
        bqr = nc.dram_tensor("bqr", [1, INNER], BF16, kind="ExternalInput")
        bkr = nc.dram_tensor("bkr", [1, INNER], BF16, kind="ExternalInput")
        bvr = nc.dram_tensor("bvr", [1, INNER], BF16, kind="ExternalInput")
    out = nc.dram_tensor("out", [BLK, D], F32, kind="ExternalOutput")

    # partition-major views for blocked DMA loads
    xTv = xT.ap().rearrange("(t p) l -> p t l", p=128)  # [128, 16, KC]
    wqv = wqT.ap().rearrange("(t p) e -> p t e", p=128)
    wkv = wkT.ap().rearrange("(t p) e -> p t e", p=128)
    wvv = wvT.ap().rearrange("(t p) e -> p t e", p=128)
    wov = woT.ap().rearrange("(t p) d -> p t d", p=128)  # [128, 8, D]

    with tile.TileContext(nc) as tc, ExitStack() as ctx:
        # ---- pools alive for the whole kernel ----
        consts = ctx.enter_context(tc.tile_pool(name="consts", bufs=1))
        big = ctx.enter_context(tc.tile_pool(name="big", bufs=1))

        ident = consts.tile([128, 128], F32, tag="ident", name="ident")
        make_identity(nc, ident)
        eps_t = consts.tile([128, 1], F32, tag="eps", name="eps")
        nc.vector.memset(eps_t, EPS)
        if with_bias:
            ones_row = consts.tile([1, 128], BF16, tag="ones_row", name="ones_row")
            nc.vector.memset(ones_row, 1.0)
            b_rows = {}
            for nm, dram in (("q", bqr), ("k", bkr), ("v", bvr)):
                b_rows[nm] = consts.tile([1, INNER], BF16, tag=f"b_{nm}", name=f"b_{nm}")
                nc.sync.dma_start(out=b_rows[nm], in_=dram.ap())

        # persistent big tiles (~96.5 KB/partition)
        kT = [big.tile([128, KC], F32R, tag=f"kT{i}", name=f"kT{i}") for i in range(NET)]
        qT = [big.tile([128, BLK], F32R, tag=f"qT{i}", name=f"qT{i}") for i in range(NET)]
        vA = [big.tile([128, VW], F32R, tag=f"vA{i}", name=f"vA{i}") for i in range(NLK)]
        aT = [big.tile([128, BLK], BF16, tag=f"aT{i}", name=f"aT{i}") for i in range(NET)]

        # ---- projection-phase pools (released before attention) ----
        pctx = ctx.enter_context(ExitStack())
        wp = pctx.enter_context(tc.tile_pool(name="wp", bufs=1))
        xp = pctx.enter_context(tc.tile_pool(name="xp", bufs=1))
        tabs = pctx.enter_context(tc.tile_pool(name="tabs", bufs=1))
        work = pctx.enter_context(tc.tile_pool(name="work", bufs=1))
        psP = pctx.enter_context(tc.tile_pool(name="psP", bufs=1, space="PSUM"))
        psT = pctx.enter_context(tc.tile_pool(name="psT", bufs=1, space="PSUM"))
        xb = 1 if with_bias else 2

        # ---------------- projections ----------------
        def make_wgs(wview):
            wgs = []
            for ec in range(NE2):
                wg = wp.tile([128, ND, 512], BF16, tag="wg", bufs=4, name="wg")
                nc.sync.dma_start(out=wg, in_=wview[:, :, ec * 512 : (ec + 1) * 512])
                wgs.append(wg)
            return wgs

        def accum_proj(xg, wgs, ec, bias_key):
            """One [128, 512] psum: x-tile block times a weight e-chunk."""
            ps = psP.tile([128, 512], F32, tag="pp", bufs=6, name="pp")
            for d in range(ND):
                nc.tensor.matmul(
                    ps,
                    lhsT=xg[:, d, :],
                    rhs=wgs[ec][:, d, :],
                    start=(d == 0),
                    stop=(d == ND - 1 and not with_bias),
                )
            if with_bias:
                nc.tensor.matmul(
                    ps,
                    lhsT=ones_row,
                    rhs=b_rows[bias_key][:, ec * 512 : (ec + 1) * 512],
                    start=False,
                    stop=True,
                )
            return ps

        def load_xg(col0, lt):
            csl = slice(col0 + lt * 128, col0 + (lt + 1) * 128)
            xg = xp.tile([128, ND, 128], BF16, tag="xg", bufs=(2 if with_bias else 3), name="xg")
            nc.sync.dma_start(out=xg, in_=xTv[:, :, csl])
            return xg

        def norm_rope_transpose(lt, ps0, ps1, tcos, tsin, dest):
            """rmsnorm + rope on a projected l-tile -> transpose into dest.

            The psum halves are copied to SBUF immediately so the PE's psum
            slots free up after ~1us instead of being held through the whole
            DVE/ACT chain (which was starving the PE of accumulation banks).
            """
            tc_t = tabs.tile([128, INNER], F32, tag="tc", bufs=2, name="tc")
            ts_t = tabs.tile([128, INNER], F32, tag="ts", bufs=2, name="ts")
            nc.sync.dma_start(out=tc_t, in_=tcos.ap()[lt * 128 : (lt + 1) * 128])
            nc.sync.dma_start(out=ts_t, in_=tsin.ap()[lt * 128 : (lt + 1) * 128])
            praw = work.tile([128, INNER], F32, tag="praw", bufs=(2 if with_bias else 3), name="praw")
            nc.scalar.copy(praw[:, 0:512], ps0)
            nc.scalar.copy(praw[:, 512:1024], ps1)
            # sum of squares (ACT Square with free-dim accumulate into ss)
            ss = work.tile([128, 2], F32, tag="ss", bufs=2, name="ss")
            for ec in range(NE2):
                sq = work.tile([128, 512], F32, tag="ropetmp", bufs=3, name="sq")
                nc.scalar.activation(
                    sq, praw[:, ec * 512 : (ec + 1) * 512],
                    mybir.ActivationFunctionType.Square,
                    accum_out=ss[:, ec : ec + 1],
                )
            rs = work.tile([128, 1], F32, tag="rs", bufs=2, name="rs")
            nc.vector.tensor_add(rs, ss[:, 0:1], ss[:, 1:2])
            nc.scalar.activation(
                rs, rs, mybir.ActivationFunctionType.Sqrt,
                bias=eps_t, scale=1.0 / INNER,
            )
            nc.vector.reciprocal(rs, rs)
            # rope: kn = (raw*cos + swap(raw)*sin') * rs    (signs folded in tables)
            kn = work.tile([128, INNER], F32, tag="kn", bufs=2, name="kn")
            for ec in range(NE2):
                h0 = ec * 512
                ph = praw[:, h0 : h0 + 512]
                knh = kn[:, h0 : h0 + 512]
                tch = tc_t[:, h0 : h0 + 512]
                tsh = ts_t[:, h0 : h0 + 512]
                tmp = work.tile([128, 512], F32, tag="ropetmp", bufs=3, name="ropetmp")
                nc.vector.tensor_mul(tmp[:, 0::2], ph[:, 1::2], tsh[:, 0::2])
                nc.vector.tensor_mul(tmp[:, 1::2], ph[:, 0::2], tsh[:, 1::2])
                nc.vector.tensor_mul(knh, ph, tch)
                nc.vector.tensor_add(knh, knh, tmp)
            nc.vector.tensor_scalar_mul(kn, kn, rs)
            # transpose [l, e] -> [e, l] into dest e-tiles
            for et in range(NET):
                pst = psT.tile([128, 128], F32, tag="pst", bufs=2, name="pst")
                nc.tensor.transpose(pst, kn[:, et * 128 : (et + 1) * 128], ident)
                nc.scalar.copy(dest[et][:, lt * 128 : (lt + 1) * 128], pst)

        # k+q pass: one x-tile load feeds k (all cols) and q (own cols, lt>=4).
        # v runs LAST so the attention scores/exp (ACT/DVE-heavy) overlap the
        # v-projection's PE work instead of serializing behind it.
        vA_r = [v.rearrange("p (h c) -> p h c", c=DH + 1) for v in vA]
        ones16 = work.tile([128, H], F32, tag="ones16", name="ones16")
        nc.vector.memset(ones16, 1.0)
        wk_gs = make_wgs(wkv)
        wq_gs = make_wgs(wqv)
        for lt in range(NLK):
            xg = load_xg(0, lt)
            kps = [accum_proj(xg, wk_gs, ec, "k") for ec in range(NE2)]
            norm_rope_transpose(lt, kps[0], kps[1], ktc, kts, kT)
            if lt >= NLK - NLQ:
                qlt = lt - (NLK - NLQ)
                qps = [accum_proj(xg, wq_gs, ec, "q") for ec in range(NE2)]
                norm_rope_transpose(qlt, qps[0], qps[1], qtc, qts, qT)

        # v pass
        wv_gs = make_wgs(wvv)
        for lt in range(NLK):
            xg = load_xg(0, lt)
            vps = [accum_proj(xg, wv_gs, ec, "v") for ec in range(NE2)]
            vct = work.tile([128, 1], F32, tag="vct", bufs=2, name="vct")
            nc.sync.dma_start(out=vct, in_=vcl.ap()[lt].rearrange("(p o) -> p o", o=1))
            nc.vector.tensor_scalar_mul(vA[lt][:, DH :: DH + 1], ones16, vct)
            for ec, ps in enumerate(vps):
                nc.scalar.copy(
                    vA_r[lt][:, ec * 8 : (ec + 1) * 8, 0:DH],
                    ps.rearrange("p (h c) -> p h c", c=DH),
                )

        # ---- release projection pools; open attention/output pools ----
        pctx.close()
        mkp = ctx.enter_context(tc.tile_pool(name="mkp", bufs=1))
        esp = ctx.enter_context(tc.tile_pool(name="esp", bufs=1))
        awork = ctx.enter_context(tc.tile_pool(name="awork", bufs=1))
        wp2 = ctx.enter_context(tc.tile_pool(name="wp2", bufs=1))
        outp = ctx.enter_context(tc.tile_pool(name="outp", bufs=1))
        psS = ctx.enter_context(tc.tile_pool(name="psS", bufs=1, space="PSUM"))
        psO = ctx.enter_context(tc.tile_pool(name="psO", bufs=1, space="PSUM"))
        psP2 = ctx.enter_context(tc.tile_pool(name="psP2", bufs=1, space="PSUM"))

        mk = [mkp.tile([128, BLK], BF16, tag=f"mk{i}", name=f"mk{i}") for i in range(NLK)]
        for t in range(NLK):
            nc.sync.dma_start(out=mk[t], in_=msk.ap()[t])

        # ---------------- attention ----------------
        # (tile, query-half) pairs: half 0 = queries [0,256), half 1 = [256,512).
        # Valid pairs cover the causal+window+sink band; SKIP_MASK pairs are
        # fully inside the band for every core (edge cores neutralized by the
        # vcl ones-column scaling), so their exp output needs no mask multiply.
        H0_T = (0, 1, 2, 3, 4, 5)
        H1_T = (0, 2, 3, 4, 5, 6, 7)
        SKIP_MASK = {(2, 0), (3, 0), (4, 1), (5, 1)}
        HALF = BLK // 2
        for h in range(H):
            et, r0 = h // 2, (h % 2) * 64
            es_tiles = {}
            pss_t = {}
            for t in range(NLK):
                halves = [hf for hf, ts in ((0, H0_T), (1, H1_T)) if t in ts]
                pss = psS.tile([128, BLK], F32, tag="pss", bufs=3, name="pss")
                es = esp.tile([128, BLK], F32R, tag="es", bufs=10, name="es")
                for hf in halves:
                    hs = slice(hf * HALF, (hf + 1) * HALF)
                    nc.tensor.matmul(
                        pss[:, hs],
                        lhsT=kT[et][r0 : r0 + 64, t * 128 : (t + 1) * 128],
                        rhs=qT[et][r0 : r0 + 64, hs],
                        start=True,
                        stop=True,
                        skip_group_check=True,
                    )
                if len(halves) == 2:
                    nc.scalar.activation(es, pss, mybir.ActivationFunctionType.Exp)
                else:
                    hs = slice(halves[0] * HALF, (halves[0] + 1) * HALF)
                    nc.scalar.activation(es[:, hs], pss[:, hs], mybir.ActivationFunctionType.Exp)
                for hf in halves:
                    if (t, hf) in SKIP_MASK:
                        continue
                    hs = slice(hf * HALF, (hf + 1) * HALF)
                    nc.vector.tensor_mul(es[:, hs], es[:, hs], mk[t][:, hs])
                es_tiles[t] = es
            pso = psO.tile([DH + 1, BLK], F32, tag="pso", bufs=2, name="pso")
            for hf, ts in ((0, H0_T), (1, H1_T)):
                hs = slice(hf * HALF, (hf + 1) * HALF)
                for i, t in enumerate(ts):
                    nc.tensor.matmul(
                        pso[:, hs],
                        lhsT=vA[t][:, h * (DH + 1) : (h + 1) * (DH + 1)],
                        rhs=es_tiles[t][:, hs],
                        start=(i == 0),
                        stop=(i == len(ts) - 1),
                        skip_group_check=True,
                    )
            rec = awork.tile([1, BLK], F32, tag="rec", bufs=2, name="rec")
            nc.vector.reciprocal(rec, pso[DH : DH + 1, :])
            rb = awork.tile([64, BLK], F32, tag="rb", bufs=2, name="rb")
            nc.gpsimd.partition_broadcast(rb, rec)
            nc.vector.tensor_mul(aT[et][r0 : r0 + 64, :], pso[0:DH, :], rb)

        # ---------------- output projection ----------------
        for dc in range(NDC):
            wog = wp2.tile([128, NET, 512], BF16, tag="wo", bufs=2, name="wo")
            nc.sync.dma_start(out=wog, in_=wov[:, :, dc * 512 : (dc + 1) * 512])
            for lt in range(NLQ):
                po = psP2.tile([128, 512], F32, tag="po", bufs=2, name="po")
                for et in range(NET):
                    nc.tensor.matmul(
                        po,
                        lhsT=_mm(aT[et][:, lt * 128 : (lt + 1) * 128]),
                        rhs=_mm(wog[:, et, :]),
                        start=(et == 0),
                        stop=(et == NET - 1),
                    )
                osb = outp.tile([128, 512], F32, tag="osb", bufs=3, name="osb")
                nc.scalar.copy(osb, po)
                nc.sync.dma_start(
                    out=out.ap()[lt * 128 : (lt + 1) * 128, dc * 512 : (dc + 1) * 512],
                    in_=osb,
                )
    nc.compile()
    return nc


_NC_CACHE = {}


def get_nc(with_bias: bool):
    if with_bias not in _NC_CACHE:
        _NC_CACHE[with_bias] = _build(with_bias)
    return _NC_CACHE[with_bias]


def _fold_tables(cosf, sinf, w):
    """Fold rmsnorm weight w (per channel) into interleaved-rope cos/sin tables.

    rope(w*q)[2i]   = w[2i]q[2i]cos[2i]     - w[2i+1]q[2i+1]sin[2i]
    rope(w*q)[2i+1] = w[2i+1]q[2i+1]cos[2i+1] + w[2i]q[2i]sin[2i+1]
    Our kernel computes raw*tc + swap(raw)*ts, so:
      tc[:, e]    = cos[:, e]    * w[e]
      ts[:, 2i]   = -sin[:, 2i]  * w[2i+1]
      ts[:, 2i+1] =  sin[:, 2i+1]* w[2i]
    """
    tc_ = cosf * w[None, :]
    ts_ = np.empty_like(sinf)
    ts_[:, 0::2] = -sinf[:, 0::2] * w[None, 1::2]
    ts_[:, 1::2] = sinf[:, 1::2] * w[None, 0::2]
    return tc_, ts_


def pack_core(c, x, cosf, sinf, lls, qn_w, kn_w, Wq, Wk, Wv, Wo, bq, bk, bv, with_bias):
    b, blk = divmod(c, NBLK)
    g0 = blk * BLK

    xTc = np.zeros((D, KC), np.float32)
    xTc[:, BLK:] = x[b, g0 : g0 + BLK].T
    xTc[:, 0] = x[b, 0]
    if blk > 0:
        xTc[:, 1:BLK] = x[b, g0 - (BLK - 1) : g0].T

    # global key position per column; validity
    glob = np.empty(KC, np.int64)
    glob[0] = 0
    glob[1:BLK] = np.arange(g0 - (BLK - 1), g0)
    glob[BLK:] = np.arange(g0, g0 + BLK)
    valid = np.ones(KC, bool)
    if blk == 0:
        valid[:BLK] = False  # sink+halo columns duplicated/invalid for block 0

    # k tables indexed by column
    ktc_f, kts_f = _fold_tables(cosf, sinf, kn_w)
    ktc_c = np.zeros((KC, INNER), np.float32)
    kts_c = np.zeros((KC, INNER), np.float32)
    gv = glob[valid]
    ktc_c[valid] = ktc_f[gv]
    kts_c[valid] = kts_f[gv]

    # q tables with logit scale and 1/sqrt(dh) folded
    qtc_f, qts_f = _fold_tables(cosf, sinf, qn_w)
    qs = (lls[g0 : g0 + BLK] * (1.0 / np.sqrt(DH)))[:, None].astype(np.float32)
    qtc_c = qtc_f[g0 : g0 + BLK] * qs
    qts_c = qts_f[g0 : g0 + BLK] * qs

    # masks [NLK, 128, BLK]
    key = glob.reshape(NLK, 128)[:, :, None]  # [t, p, 1]
    q_pos = (g0 + np.arange(BLK))[None, None, :]  # [1, 1, f]
    m = (
        valid.reshape(NLK, 128)[:, :, None]
        & (key <= q_pos)
        & ((key > q_pos - WINDOW) | (key < SINK))
    ).astype(np.float32)

    im = {
        "xT": xTc.astype(ml_dtypes.bfloat16),
        "wqT": np.ascontiguousarray(Wq.T).astype(ml_dtypes.bfloat16),
        "wkT": np.ascontiguousarray(Wk.T).astype(ml_dtypes.bfloat16),
        "wvT": np.ascontiguousarray(Wv.T).astype(ml_dtypes.bfloat16),
        "woT": np.ascontiguousarray(Wo.T).astype(ml_dtypes.bfloat16),
        "qtc": qtc_c,
        "qts": qts_c,
        "ktc": ktc_c,
        "kts": kts_c,
        "msk": m.astype(ml_dtypes.bfloat16),
        "vcl": valid.reshape(NLK, 128).astype(np.float32),
    }
    if with_bias:
        im["bqr"] = bq[None, :].astype(ml_dtypes.bfloat16)
        im["bkr"] = bk[None, :].astype(ml_dtypes.bfloat16)
        im["bvr"] = bv[None, :].astype(ml_dtypes.bfloat16)
    return im


def make_in_maps(inputs):
    f = lambda k: np.asarray(inputs[k], np.float32)
    x = f("x")
    cosf = f("cos")[0]
    sinf = f("sin")[0]
    lls = f("logit_log_scale")[0, :, 0]
    bq, bk, bv = f("bq"), f("bk"), f("bv")
    with_bias = bool(np.any(bq) or np.any(bk) or np.any(bv))
    ims = [
        pack_core(
            c, x, cosf, sinf, lls, f("qn_w"), f("kn_w"),
            f("Wq"), f("Wk"), f("Wv"), f("Wo"), bq, bk, bv, with_bias,
        )
        for c in range(NCORES)
    ]
    return ims, with_bias


last_results = None


def kernel(**inputs):
    global last_results
    ims, with_bias = make_in_maps(inputs)
    nc = get_nc(with_bias)
    res = run_bass_kernel_spmd(nc, ims, core_ids=list(range(NCORES)))
    last_results = res
    out = np.empty((B, L, D), np.float32)
    for c, om in enumerate(res.results):
        b, blk = divmod(c, NBLK)
        out[b, blk * BLK : (blk + 1) * BLK, :] = om["out"]
    out += np.asarray(inputs["bo"], np.float32)[None, None, :]
    return out



# revision 20
# speedup vs baseline: 1.3011x; 1.3011x over previous
"""Trainium2 Bass kernel for nn_CausalLTXAttention (sliding-window + sink causal attention).

Sharding: 8 cores = 2 batches x 4 sequence blocks of 512 queries each.
Each core computes QKV projections for its block (+ a 511-key halo + the sink
key 0), RMSNorm + interleaved RoPE (norm weights / logit scale / 1/sqrt(dh)
folded into host-precomputed cos/sin tables), banded-causal attention with
max-free softmax (scores bounded ~7.5), and the output projection for its own
512 rows.  Outputs are disjoint -> host just concatenates (and adds bo).

v2 layout/engine notes:
  - QKV projections run in fp8(e4m3) with DoubleRow perf mode (2 contraction
    tiles per matmul).  Weights are host-prescaled by 64 so they sit in the
    e4m3 normal range; the 64x cancels in the q/k rmsnorm scale and is folded
    out of Wo for the v path.
  - The per-head channel layout for q/k is host-permuted to [evens|odds] 32
    blocks, so the RoPE swap becomes stride-1 block ops (DVE 2x bf16 mode).
    Scores are permutation-invariant per head; v stays unpermuted.
  - The k-side rmsnorm scale (one scalar per key) is folded into the exp()
    scale argument of the score softmax instead of scaling kn.
  - PSUM->SBUF copies run on the Pool engine; sum-of-squares runs on DVE
    (tensor_tensor_reduce); ACT does only Sqrt + the attention exp()s.
  - x and the QKV weights load once up-front (d-split for a fast start).
"""

import os
from contextlib import ExitStack

import numpy as np
import ml_dtypes

import concourse.bass as bass
import concourse.bacc as bacc
import concourse.mybir as mybir
import concourse.tile as tile
from concourse.bass_utils import run_bass_kernel_spmd
from concourse.masks import make_identity

# ---- problem constants (hardcoded per the harness contract) ----
B, L, D = 2, 2048, 2048
H, DH = 16, 64
INNER = H * DH  # 1024
WINDOW, SINK = 512, 1
EPS = 1e-6
NCORES, NBLK = 8, 4
BLK = L // NBLK  # 512 queries per core
KC = 1024  # key cols per core: [sink | halo 511 | own 512]
NLK = KC // 128  # 8 key l-tiles
NLQ = BLK // 128  # 4 own l-tiles
ND = D // 128  # 16 contraction d-tiles
NDP = ND // 2  # 8 DoubleRow d-pairs
NE2 = INNER // 512  # 2 e-chunks for projections
NET = INNER // 128  # 8 e-tiles
NDC = D // 512  # 4 output d-chunks
VW = H * (DH + 1)  # 1040: v tiles with a ones column per head
WSCALE = 64.0  # fp8 weight prescale

F32 = mybir.dt.float32
BF16 = mybir.dt.bfloat16
FP8 = mybir.dt.float8e4
DR = mybir.MatmulPerfMode.DoubleRow
MULT = mybir.AluOpType.mult
ADD = mybir.AluOpType.add

PROJ_FP8 = os.environ.get("PROJ_DT", "fp8") == "fp8"
NO_DR = bool(os.environ.get("NO_DR"))  # fp8 without DoubleRow perf mode
NO_EXPSCALE = bool(os.environ.get("NO_EXPSCALE"))  # rs on kn instead of exp scale
NO_POOLMASK = bool(os.environ.get("NO_POOLMASK"))  # mask muls on DVE
NO_BF16T = bool(os.environ.get("NO_BF16T"))  # f32 rope output + transposes


def _build(with_bias: bool):
    nc = bacc.Bacc("TRN2", target_bir_lowering=False, debug=False)
    wdt = FP8 if PROJ_FP8 else BF16

    x8 = nc.dram_tensor("x8", [D, KC], wdt, kind="ExternalInput")
    wq8 = nc.dram_tensor("wq8", [D, INNER], wdt, kind="ExternalInput")
    wk8 = nc.dram_tensor("wk8", [D, INNER], wdt, kind="ExternalInput")
    wv8 = nc.dram_tensor("wv8", [D, INNER], wdt, kind="ExternalInput")
    woT = nc.dram_tensor("woT", [INNER, D], BF16, kind="ExternalInput")
    qtc = nc.dram_tensor("qtc", [BLK, INNER], BF16, kind="ExternalInput")
    qts = nc.dram_tensor("qts", [BLK, INNER], BF16, kind="ExternalInput")
    ktc = nc.dram_tensor("ktc", [KC, INNER], BF16, kind="ExternalInput")
    kts = nc.dram_tensor("kts", [KC, INNER], BF16, kind="ExternalInput")
    msk = nc.dram_tensor("msk", [NLK, 128, BLK], BF16, kind="ExternalInput")
    vcl = nc.dram_tensor("vcl", [NLK, 128], F32, kind="ExternalInput")
    if with_bias:
        bqr = nc.dram_tensor("bqr", [1, INNER], BF16, kind="ExternalInput")
        bkr = nc.dram_tensor("bkr", [1, INNER], BF16, kind="ExternalInput")
        bvr = nc.dram_tensor("bvr", [1, INNER], BF16, kind="ExternalInput")
    out = nc.dram_tensor("out", [BLK, D], F32, kind="ExternalOutput")

    # partition-major views for blocked DMA loads
    x8v = x8.ap().rearrange("(t p) l -> p t l", p=128)  # [128, 16, KC]
    wqv = wq8.ap().rearrange("(t p) e -> p t e", p=128)
    wkv = wk8.ap().rearrange("(t p) e -> p t e", p=128)
    wvv = wv8.ap().rearrange("(t p) e -> p t e", p=128)
    wov = woT.ap().rearrange("(t p) d -> p t d", p=128)  # [128, 8, D]

    with tile.TileContext(nc) as tc, ExitStack() as ctx:
        # ---- pools alive for the whole kernel ----
        consts = ctx.enter_context(tc.tile_pool(name="consts", bufs=1))
        big = ctx.enter_context(tc.tile_pool(name="big", bufs=1))

        ident = consts.tile([128, 128], F32 if NO_BF16T else BF16, tag="ident", name="ident")
        make_identity(nc, ident)
        eps_t = consts.tile([128, 1], F32, tag="eps", name="eps")
        sscale = 1.0 / INNER
        nc.vector.memset(eps_t, EPS * (WSCALE * WSCALE if PROJ_FP8 else 1.0))
        ones16 = consts.tile([128, H], F32, tag="ones16", name="ones16")
        nc.vector.memset(ones16, 1.0)
        if with_bias:
            ones_row = consts.tile([1, 128], BF16, tag="ones_row", name="ones_row")
            nc.vector.memset(ones_row, 1.0)
            b_rows = {}
            for nm, dram in (("q", bqr), ("k", bkr), ("v", bvr)):
                b_rows[nm] = consts.tile([1, INNER], BF16, tag=f"b_{nm}", name=f"b_{nm}")
                nc.sync.dma_start(out=b_rows[nm], in_=dram.ap())

        # persistent big tiles (per-partition: x 16K(fp8), w 3x16K(fp8),
        # kT 16K, qT 8K, vA 16.25K, aT 8K bf16)
        xg = big.tile([128, ND, KC], wdt, tag="xg", name="xg")
        wgt = {}
        for nm, view in (("q", wqv), ("k", wkv), ("v", wvv)):
            wgt[nm] = big.tile([128, ND, INNER], wdt, tag=f"w_{nm}", name=f"w_{nm}")
        kT = big.tile([128, NET, KC], BF16, tag="kT", name="kT")
        qT = big.tile([128, NET, BLK], BF16, tag="qT", name="qT")
        vA = [big.tile([128, VW], BF16, tag=f"vA{i}", name=f"vA{i}") for i in range(NLK)]
        vA_r = [v.rearrange("p (h c) -> p h c", c=DH + 1) for v in vA]
        aT = big.tile([128, NET, BLK], BF16, tag="aT", name="aT")
        # k-side rmsnorm scales, consumed by the attention exp()
        rsk = [consts.tile([128, 1], F32, tag=f"rsk{i}", name=f"rsk{i}") for i in range(NLK)]
        mk = [consts.tile([128, BLK], BF16, tag=f"mk{i}", name=f"mk{i}") for i in range(NLK)]

        # ---- up-front loads, d-split so the first matmuls start early ----
        hd = ND // 2
        nc.sync.dma_start(out=xg[:, 0:hd, BLK:KC], in_=x8v[:, 0:hd, BLK:KC])
        for nm in ("q", "k", "v"):
            view = {"q": wqv, "k": wkv, "v": wvv}[nm]
            nc.sync.dma_start(out=wgt[nm][:, 0:hd, :], in_=view[:, 0:hd, :])
            nc.sync.dma_start(out=wgt[nm][:, hd:ND, :], in_=view[:, hd:ND, :])
        nc.sync.dma_start(out=xg[:, hd:ND, BLK:KC], in_=x8v[:, hd:ND, BLK:KC])
        nc.sync.dma_start(out=xg[:, 0:hd, 0:BLK], in_=x8v[:, 0:hd, 0:BLK])
        nc.sync.dma_start(out=xg[:, hd:ND, 0:BLK], in_=x8v[:, hd:ND, 0:BLK])
        for t in range(NLK):
            nc.sync.dma_start(out=mk[t], in_=msk.ap()[t])

        # ---- projection-phase pools (released before attention) ----
        pctx = ctx.enter_context(ExitStack())
        tabs = pctx.enter_context(tc.tile_pool(name="tabs", bufs=1))
        work = pctx.enter_context(tc.tile_pool(name="work", bufs=1))
        psP = pctx.enter_context(tc.tile_pool(name="psP", bufs=1, space="PSUM"))
        psT = pctx.enter_context(tc.tile_pool(name="psT", bufs=1, space="PSUM"))

        def proj_ps(w, lt, bias_key):
            """[128, 1024] psum tile: x l-tile times the full weight."""
            ps = psP.tile([128, INNER], F32, tag="pp", bufs=3, name="pp")
            for ec in range(NE2):
                es_ = slice(ec * 512, (ec + 1) * 512)
                if PROJ_FP8 and not NO_DR:
                    for dp in range(NDP):
                        nc.tensor.matmul(
                            ps[:, es_],
                            lhsT=xg[:, 2 * dp : 2 * dp + 2, lt * 128 : (lt + 1) * 128],
                            rhs=w[:, 2 * dp : 2 * dp + 2, es_],
                            start=(dp == 0),
                            stop=(dp == NDP - 1 and not with_bias),
                            perf_mode=DR,
                        )
                else:
                    for d in range(ND):
                        nc.tensor.matmul(
                            ps[:, es_],
                            lhsT=xg[:, d, lt * 128 : (lt + 1) * 128],
                            rhs=w[:, d, es_],
                            start=(d == 0),
                            stop=(d == ND - 1 and not with_bias),
                        )
                if with_bias:
                    nc.tensor.matmul(
                        ps[:, es_],
                        lhsT=ones_row,
                        rhs=b_rows[bias_key][:, es_],
                        start=False,
                        stop=True,
                    )
            return ps

        def rms_scale(praw, ss):
            """rs = 1/(WSCALE*sqrt(mean_sq_true + eps)) as a [128,1] tile."""
            rs = work.tile([128, 1], F32, tag="rs", bufs=4, name="rs")
            nc.scalar.activation(
                rs, ss, mybir.ActivationFunctionType.Sqrt, bias=eps_t, scale=sscale
            )
            nc.vector.reciprocal(rs, rs)
            return rs

        def norm_rope_transpose(lt, ps, tcos, tsin, dest, dest_w, q_side):
            """rmsnorm + rope on a projected l-tile -> transpose into dest.

            dest is a [128, NET, W] big tile; returns the k-side rs (folded
            into exp() later) or None for q (rs applied to kn here).
            """
            kdt = F32 if NO_BF16T else BF16
            tc_t = tabs.tile([128, INNER], BF16, tag="tc", bufs=3, name="tc")
            ts_t = tabs.tile([128, INNER], BF16, tag="ts", bufs=3, name="ts")
            nc.sync.dma_start(out=tc_t, in_=tcos.ap()[lt * 128 : (lt + 1) * 128])
            nc.sync.dma_start(out=ts_t, in_=tsin.ap()[lt * 128 : (lt + 1) * 128])
            praw = work.tile([128, INNER], BF16, tag="praw", bufs=3, name="praw")
            # per-512 copies: a single ACT access must not cross a PSUM bank
            nc.scalar.copy(praw[:, 0:512], ps[:, 0:512])
            nc.scalar.copy(praw[:, 512:1024], ps[:, 512:1024])
            ss = work.tile([128, 1], F32, tag="ss", bufs=4, name="ss")
            sqd = work.tile([128, INNER], BF16, tag="sqd", bufs=2, name="sqd")
            nc.vector.tensor_tensor_reduce(
                out=sqd, in0=praw, in1=praw, scale=1.0, scalar=0.0,
                op0=MULT, op1=ADD, accum_out=ss,
            )
            rs = rms_scale(praw, ss)
            # rope in the [evens|odds] block layout: swap = exchange 32-blocks
            kn = work.tile([128, INNER], kdt, tag="kn", bufs=3, name="kn")
            tmp = work.tile([128, INNER], kdt, tag="tmp", bufs=2, name="tmp")
            p4 = praw.rearrange("p (g t c) -> p g t c", t=2, c=32)
            t4 = tmp.rearrange("p (g t c) -> p g t c", t=2, c=32)
            s4 = ts_t.rearrange("p (g t c) -> p g t c", t=2, c=32)
            if NO_EXPSCALE and not q_side:
                q_side = True  # apply rs to kn on DVE instead of at exp()
            if q_side:
                nc.vector.scalar_tensor_tensor(
                    out=t4[:, :, 0, :], in0=p4[:, :, 1, :], scalar=rs,
                    in1=s4[:, :, 0, :], op0=MULT, op1=MULT,
                )
                nc.vector.scalar_tensor_tensor(
                    out=t4[:, :, 1, :], in0=p4[:, :, 0, :], scalar=rs,
                    in1=s4[:, :, 1, :], op0=MULT, op1=MULT,
                )
                nc.vector.scalar_tensor_tensor(
                    out=kn, in0=praw, scalar=rs, in1=tc_t, op0=MULT, op1=MULT,
                )
            else:
                nc.vector.tensor_mul(t4[:, :, 0, :], p4[:, :, 1, :], s4[:, :, 0, :])
                nc.vector.tensor_mul(t4[:, :, 1, :], p4[:, :, 0, :], s4[:, :, 1, :])
                nc.vector.tensor_mul(kn, praw, tc_t)
            nc.vector.tensor_add(kn, kn, tmp)
            # transpose [l, e] -> [e, l] into dest e-tiles (2x 4-tile groups)
            for g in range(2):
                pst = psT.tile([128, 512], kdt, tag="pst", bufs=2, name="pst")
                for i in range(4):
                    et = g * 4 + i
                    nc.tensor.transpose(
                        pst[:, i * 128 : (i + 1) * 128],
                        kn[:, et * 128 : (et + 1) * 128],
                        ident,
                    )
                nc.vector.tensor_copy(
                    dest[:, g * 4 : (g + 1) * 4, lt * 128 : (lt + 1) * 128],
                    pst.rearrange("p (i c) -> p i c", c=128),
                )
            return rs

        kTr = kT  # [128, NET, KC]
        qTr = qT  # [128, NET, BLK]

        # k pass (all 8 l-tiles) + q pass (own 4 l-tiles) share the x tile.
        for lt in range(NLK):
            kps = proj_ps(wgt["k"], lt, "k")
            rs = norm_rope_transpose(lt, kps, ktc, kts, kTr, KC, q_side=False)
            nc.vector.tensor_copy(rsk[lt], rs)
            if lt >= NLK - NLQ:
                qlt = lt - (NLK - NLQ)
                qps = proj_ps(wgt["q"], qlt + 4, "q")
                norm_rope_transpose(qlt, qps, qtc, qts, qTr, BLK, q_side=True)

        # v pass
        for lt in range(NLK):
            vps = proj_ps(wgt["v"], lt, "v")
            vct = work.tile([128, 1], F32, tag="vct", bufs=2, name="vct")
            nc.sync.dma_start(out=vct, in_=vcl.ap()[lt].rearrange("(p o) -> p o", o=1))
            nc.gpsimd.tensor_scalar_mul(vA[lt][:, DH :: DH + 1], ones16, vct)
            for ec in range(NE2):
                nc.vector.tensor_copy(
                    vA_r[lt][:, ec * 8 : (ec + 1) * 8, 0:DH],
                    vps[:, ec * 512 : (ec + 1) * 512].rearrange("p (h c) -> p h c", c=DH),
                )

        # ---- release projection pools; open attention/output pools ----
        pctx.close()
        esp = ctx.enter_context(tc.tile_pool(name="esp", bufs=1))
        awork = ctx.enter_context(tc.tile_pool(name="awork", bufs=1))
        wp2 = ctx.enter_context(tc.tile_pool(name="wp2", bufs=1))
        outp = ctx.enter_context(tc.tile_pool(name="outp", bufs=1))
        psS = ctx.enter_context(tc.tile_pool(name="psS", bufs=1, space="PSUM"))
        psO = ctx.enter_context(tc.tile_pool(name="psO", bufs=1, space="PSUM"))
        psP2 = ctx.enter_context(tc.tile_pool(name="psP2", bufs=1, space="PSUM"))

        # ---------------- attention ----------------
        # (tile, query-half) pairs: half 0 = queries [0,256), half 1 = [256,512).
        # Valid pairs cover the causal+window+sink band; SKIP_MASK pairs are
        # fully inside the band for every core (edge cores neutralized by the
        # vcl ones-column scaling), so their exp output needs no mask multiply.
        H0_T = (0, 1, 2, 3, 4, 5)
        H1_T = (0, 2, 3, 4, 5, 6, 7)
        SKIP_MASK = {(2, 0), (3, 0), (4, 1), (5, 1)}
        HALF = BLK // 2
        for h in range(H):
            et, r0 = h // 2, (h % 2) * 64
            es_tiles = {}
            for t in range(NLK):
                halves = [hf for hf, ts_ in ((0, H0_T), (1, H1_T)) if t in ts_]
                pss = psS.tile([128, BLK], F32, tag="pss", bufs=3, name="pss")
                es = esp.tile([128, BLK], BF16, tag="es", bufs=10, name="es")
                cs = slice(0, BLK) if len(halves) == 2 else slice(
                    halves[0] * HALF, (halves[0] + 1) * HALF
                )
                nc.tensor.matmul(
                    pss[:, cs],
                    lhsT=kT[r0 : r0 + 64, et, t * 128 : (t + 1) * 128],
                    rhs=qT[r0 : r0 + 64, et, cs],
                    start=True,
                    stop=True,
                    skip_group_check=True,
                )
                if NO_EXPSCALE:
                    nc.scalar.activation(
                        es[:, cs], pss[:, cs], mybir.ActivationFunctionType.Exp
                    )
                else:
                    nc.scalar.activation(
                        es[:, cs], pss[:, cs], mybir.ActivationFunctionType.Exp,
                        scale=rsk[t],
                    )
                meng = nc.vector if NO_POOLMASK else nc.gpsimd
                mh = [hf for hf in halves if (t, hf) not in SKIP_MASK]
                if len(mh) == 2:
                    meng.tensor_mul(es, es, mk[t])
                else:
                    for hf in mh:
                        hs = slice(hf * HALF, (hf + 1) * HALF)
                        meng.tensor_mul(es[:, hs], es[:, hs], mk[t][:, hs])
                es_tiles[t] = es
            pso = psO.tile([DH + 1, BLK], F32, tag="pso", bufs=2, name="pso")
            for hf, ts_ in ((0, H0_T), (1, H1_T)):
                hs = slice(hf * HALF, (hf + 1) * HALF)
                for i, t in enumerate(ts_):
                    nc.tensor.matmul(
                        pso[:, hs],
                        lhsT=vA[t][:, h * (DH + 1) : (h + 1) * (DH + 1)],
                        rhs=es_tiles[t][:, hs],
                        start=(i == 0),
                        stop=(i == len(ts_) - 1),
                        skip_group_check=True,
                    )
            rec = awork.tile([1, BLK], F32, tag="rec", bufs=2, name="rec")
            nc.vector.reciprocal(rec, pso[DH : DH + 1, :])
            rb = awork.tile([64, BLK], F32, tag="rb", bufs=2, name="rb")
            nc.gpsimd.partition_broadcast(rb, rec)
            nc.vector.tensor_mul(aT[r0 : r0 + 64, et, :], pso[0:DH, :], rb)

        # ---------------- output projection ----------------
        for dc in range(NDC):
            wog = wp2.tile([128, NET, 512], BF16, tag="wo", bufs=2, name="wo")
            nc.sync.dma_start(out=wog, in_=wov[:, :, dc * 512 : (dc + 1) * 512])
            for lt in range(NLQ):
                po = psP2.tile([128, 512], F32, tag="po", bufs=2, name="po")
                for et in range(NET):
                    nc.tensor.matmul(
                        po,
                        lhsT=aT[:, et, lt * 128 : (lt + 1) * 128],
                        rhs=wog[:, et, :],
                        start=(et == 0),
                        stop=(et == NET - 1),
                    )
                osb = outp.tile([128, 512], F32, tag="osb", bufs=3, name="osb")
                nc.scalar.copy(osb, po)
                nc.sync.dma_start(
                    out=out.ap()[lt * 128 : (lt + 1) * 128, dc * 512 : (dc + 1) * 512],
                    in_=osb,
                )
    nc.compile()
    return nc


_NC_CACHE = {}


def get_nc(with_bias: bool = False):
    if with_bias not in _NC_CACHE:
        _NC_CACHE[with_bias] = _build(with_bias)
    return _NC_CACHE[with_bias]


def _fold_tables(cosf, sinf, w):
    """Fold rmsnorm weight w (per channel) into interleaved-rope cos/sin tables.

    rope(w*q)[2i]   = w[2i]q[2i]cos[2i]     - w[2i+1]q[2i+1]sin[2i]
    rope(w*q)[2i+1] = w[2i+1]q[2i+1]cos[2i+1] + w[2i]q[2i]sin[2i+1]
    Our kernel computes raw*tc + swap(raw)*ts, so:
      tc[:, e]    = cos[:, e]    * w[e]
      ts[:, 2i]   = -sin[:, 2i]  * w[2i+1]
      ts[:, 2i+1] =  sin[:, 2i+1]* w[2i]
    """
    tc_ = cosf * w[None, :]
    ts_ = np.empty_like(sinf)
    ts_[:, 0::2] = -sinf[:, 0::2] * w[None, 1::2]
    ts_[:, 1::2] = sinf[:, 1::2] * w[None, 0::2]
    return tc_, ts_


def _perm_old_of_new():
    """Per-head [evens|odds] channel permutation: old index for each new col."""
    p = np.empty(INNER, np.int64)
    for h in range(H):
        base = h * DH
        p[base : base + 32] = base + 2 * np.arange(32)
        p[base + 32 : base + 64] = base + 2 * np.arange(32) + 1
    return p


_PERM = _perm_old_of_new()


def _wcast(a):
    if PROJ_FP8:
        return (a * WSCALE).astype(ml_dtypes.float8_e4m3)
    return a.astype(ml_dtypes.bfloat16)


def pack_core(c, x, cosf, sinf, lls, qn_w, kn_w, Wq, Wk, Wv, Wo, bq, bk, bv, with_bias):
    b, blk = divmod(c, NBLK)
    g0 = blk * BLK

    xTc = np.zeros((D, KC), np.float32)
    xTc[:, BLK:] = x[b, g0 : g0 + BLK].T
    xTc[:, 0] = x[b, 0]
    if blk > 0:
        xTc[:, 1:BLK] = x[b, g0 - (BLK - 1) : g0].T

    # global key position per column; validity
    glob = np.empty(KC, np.int64)
    glob[0] = 0
    glob[1:BLK] = np.arange(g0 - (BLK - 1), g0)
    glob[BLK:] = np.arange(g0, g0 + BLK)
    valid = np.ones(KC, bool)
    if blk == 0:
        valid[:BLK] = False  # sink+halo columns duplicated/invalid for block 0

    # k tables indexed by column (channel-permuted)
    ktc_f, kts_f = _fold_tables(cosf, sinf, kn_w)
    ktc_c = np.zeros((KC, INNER), np.float32)
    kts_c = np.zeros((KC, INNER), np.float32)
    gv = glob[valid]
    ktc_c[valid] = ktc_f[gv]
    kts_c[valid] = kts_f[gv]

    # q tables with logit scale and 1/sqrt(dh) folded
    qtc_f, qts_f = _fold_tables(cosf, sinf, qn_w)
    qs = (lls[g0 : g0 + BLK] * (1.0 / np.sqrt(DH)))[:, None].astype(np.float32)
    qtc_c = qtc_f[g0 : g0 + BLK] * qs
    qts_c = qts_f[g0 : g0 + BLK] * qs

    # masks [NLK, 128, BLK]
    key = glob.reshape(NLK, 128)[:, :, None]  # [t, p, 1]
    q_pos = (g0 + np.arange(BLK))[None, None, :]  # [1, 1, f]
    m = (
        valid.reshape(NLK, 128)[:, :, None]
        & (key <= q_pos)
        & ((key > q_pos - WINDOW) | (key < SINK))
    ).astype(np.float32)

    P = _PERM
    im = {
        "x8": _wcast_x(xTc),
        "wq8": _wcast(np.ascontiguousarray(Wq.T)[:, P]),
        "wk8": _wcast(np.ascontiguousarray(Wk.T)[:, P]),
        "wv8": _wcast(np.ascontiguousarray(Wv.T)),
        "woT": (np.ascontiguousarray(Wo.T) * (1.0 / WSCALE if PROJ_FP8 else 1.0)).astype(
            ml_dtypes.bfloat16
        ),
        "qtc": qtc_c[:, P].astype(ml_dtypes.bfloat16),
        "qts": qts_c[:, P].astype(ml_dtypes.bfloat16),
        "ktc": ktc_c[:, P].astype(ml_dtypes.bfloat16),
        "kts": kts_c[:, P].astype(ml_dtypes.bfloat16),
        "msk": m.astype(ml_dtypes.bfloat16),
        "vcl": valid.reshape(NLK, 128).astype(np.float32),
    }
    if with_bias:
        sc = WSCALE if PROJ_FP8 else 1.0
        im["bqr"] = (bq[None, P] * sc).astype(ml_dtypes.bfloat16)
        im["bkr"] = (bk[None, P] * sc).astype(ml_dtypes.bfloat16)
        im["bvr"] = (bv[None, :] * sc).astype(ml_dtypes.bfloat16)
    return im


def _wcast_x(a):
    if PROJ_FP8:
        return a.astype(ml_dtypes.float8_e4m3)
    return a.astype(ml_dtypes.bfloat16)


def make_in_maps(inputs):
    f = lambda k: np.asarray(inputs[k], np.float32)
    x = f("x")
    cosf = f("cos")[0]
    sinf = f("sin")[0]
    lls = f("logit_log_scale")[0, :, 0]
    bq, bk, bv = f("bq"), f("bk"), f("bv")
    with_bias = bool(np.any(bq) or np.any(bk) or np.any(bv))
    ims = [
        pack_core(
            c, x, cosf, sinf, lls, f("qn_w"), f("kn_w"),
            f("Wq"), f("Wk"), f("Wv"), f("Wo"), bq, bk, bv, with_bias,
        )
        for c in range(NCORES)
    ]
    return ims, with_bias


last_results = None


def kernel(**inputs):
    global last_results
    ims, with_bias = make_in_maps(inputs)
    nc = get_nc(with_bias)
    res = run_bass_kernel_spmd(nc, ims, core_ids=list(range(NCORES)))
    last_results = res
    out = np.empty((B, L, D), np.float32)
    for c, om in enumerate(res.results):
        b, blk = divmod(c, NBLK)
        out[b, blk * BLK : (blk + 1) * BLK, :] = om["out"]
    out += np.asarray(inputs["bo"], np.float32)[None, None, :]
    return out


# revision 35
# speedup vs baseline: 1.5462x; 1.1884x over previous
"""Trainium2 Bass kernel for nn_CausalLTXAttention (sliding-window + sink causal attention).

Sharding: 8 cores = 2 batches x 4 sequence blocks of 512 queries each.
Each core computes QKV projections for its block (+ a 511-key halo + the sink
key 0), RMSNorm + interleaved RoPE (norm weights / logit scale / 1/sqrt(dh)
folded into host-precomputed cos/sin tables), banded-causal attention with
max-free softmax (scores bounded ~7.5), and the output projection for its own
512 rows.  Outputs are disjoint -> host just concatenates (and adds bo).

v2 layout/engine notes:
  - QKV projections run in fp8(e4m3) with DoubleRow perf mode (2 contraction
    tiles per matmul).  Weights are host-prescaled by 64 so they sit in the
    e4m3 normal range; the 64x cancels in the q/k rmsnorm scale and is folded
    out of Wo for the v path.
  - The per-head channel layout for q/k is host-permuted to [evens|odds] 32
    blocks, so the RoPE swap becomes stride-1 block ops (DVE 2x bf16 mode).
    Scores are permutation-invariant per head; v stays unpermuted.
  - The k-side rmsnorm scale (one scalar per key) is folded into the exp()
    scale argument of the score softmax instead of scaling kn.
  - PSUM->SBUF copies run on the Pool engine; sum-of-squares runs on DVE
    (tensor_tensor_reduce); ACT does only Sqrt + the attention exp()s.
  - x and the QKV weights load once up-front (d-split for a fast start).
"""

import os
from contextlib import ExitStack

import numpy as np
import ml_dtypes

import concourse.bass as bass
import concourse.bacc as bacc
import concourse.mybir as mybir
import concourse.tile as tile
from concourse.bass_utils import run_bass_kernel_spmd
from concourse.masks import make_identity

# ---- problem constants (hardcoded per the harness contract) ----
B, L, D = 2, 2048, 2048
H, DH = 16, 64
INNER = H * DH  # 1024
WINDOW, SINK = 512, 1
EPS = 1e-6
NCORES, NBLK = 8, 4
BLK = L // NBLK  # 512 queries per core
KC = 1024  # key cols per core: [sink | halo 511 | own 512]
NLK = KC // 128  # 8 key l-tiles
NLQ = BLK // 128  # 4 own l-tiles
ND = D // 128  # 16 contraction d-tiles
NDP = ND // 2  # 8 DoubleRow d-pairs
NE2 = INNER // 512  # 2 e-chunks for projections
NET = INNER // 128  # 8 e-tiles
NDC = D // 512  # 4 output d-chunks
VW = H * (DH + 1)  # 1040: v tiles with a ones column per head
WSCALE = 64.0  # fp8 weight prescale

F32 = mybir.dt.float32
BF16 = mybir.dt.bfloat16
FP8 = mybir.dt.float8e4
DR = mybir.MatmulPerfMode.DoubleRow
MULT = mybir.AluOpType.mult
ADD = mybir.AluOpType.add

PROJ_FP8 = os.environ.get("PROJ_DT", "fp8") == "fp8"
NO_DR = bool(os.environ.get("NO_DR"))  # fp8 without DoubleRow perf mode
NO_EXPSCALE = bool(os.environ.get("NO_EXPSCALE"))  # rs on kn instead of exp scale
NO_POOLMASK = bool(os.environ.get("NO_POOLMASK"))  # mask muls on DVE
NO_BF16T = bool(os.environ.get("NO_BF16T"))  # f32 rope output + transposes


def _build(with_bias: bool):
    nc = bacc.Bacc("TRN2", target_bir_lowering=False, debug=False)
    wdt = FP8 if PROJ_FP8 else BF16

    refine = PROJ_FP8 and not NO_DR
    x8 = nc.dram_tensor("x8", [D, KC], wdt, kind="ExternalInput")
    wq8 = nc.dram_tensor("wq8", [D, INNER], wdt, kind="ExternalInput")
    wk8 = nc.dram_tensor("wk8", [D, INNER], wdt, kind="ExternalInput")
    wv8 = nc.dram_tensor("wv8", [D, INNER], wdt, kind="ExternalInput")
    if refine:
        # fp8 hi/lo residuals (x own rows 0..127; lo-weights), scaled x16
        x8l = nc.dram_tensor("x8l", [D, 128], FP8, kind="ExternalInput")
        wql = nc.dram_tensor("wql", [D, INNER], FP8, kind="ExternalInput")
        wkl = nc.dram_tensor("wkl", [D, INNER], FP8, kind="ExternalInput")
        wvl = nc.dram_tensor("wvl", [D, INNER], FP8, kind="ExternalInput")
    woT = nc.dram_tensor("woT", [INNER, D], BF16, kind="ExternalInput")
    qtc = nc.dram_tensor("qtc", [BLK, INNER], BF16, kind="ExternalInput")
    qts = nc.dram_tensor("qts", [BLK, INNER], BF16, kind="ExternalInput")
    ktc = nc.dram_tensor("ktc", [KC, INNER], BF16, kind="ExternalInput")
    kts = nc.dram_tensor("kts", [KC, INNER], BF16, kind="ExternalInput")
    msk = nc.dram_tensor("msk", [NLK, 128, BLK], BF16, kind="ExternalInput")
    vcl = nc.dram_tensor("vcl", [NLK, 128], F32, kind="ExternalInput")
    if with_bias:
        bqr = nc.dram_tensor("bqr", [1, INNER], BF16, kind="ExternalInput")
        bkr = nc.dram_tensor("bkr", [1, INNER], BF16, kind="ExternalInput")
        bvr = nc.dram_tensor("bvr", [1, INNER], BF16, kind="ExternalInput")
    out = nc.dram_tensor("out", [BLK, D], F32, kind="ExternalOutput")

    # partition-major views for blocked DMA loads
    x8v = x8.ap().rearrange("(t p) l -> p t l", p=128)  # [128, 16, KC]
    wqv = wq8.ap().rearrange("(t p) e -> p t e", p=128)
    wkv = wk8.ap().rearrange("(t p) e -> p t e", p=128)
    wvv = wv8.ap().rearrange("(t p) e -> p t e", p=128)
    wov = woT.ap().rearrange("(t p) d -> p t d", p=128)  # [128, 8, D]
    if refine:
        xlv = x8l.ap().rearrange("(t p) l -> p t l", p=128)  # [128, 16, 128]
        wlv = {
            "q": wql.ap().rearrange("(t p) e -> p t e", p=128),
            "k": wkl.ap().rearrange("(t p) e -> p t e", p=128),
            "v": wvl.ap().rearrange("(t p) e -> p t e", p=128),
        }

    with tile.TileContext(nc) as tc, ExitStack() as ctx:
        # ---- pools alive for the whole kernel ----
        consts = ctx.enter_context(tc.tile_pool(name="consts", bufs=1))
        big = ctx.enter_context(tc.tile_pool(name="big", bufs=1))

        ident = consts.tile([128, 128], F32 if NO_BF16T else BF16, tag="ident", name="ident")
        make_identity(nc, ident)
        eps_t = consts.tile([128, 1], F32, tag="eps", name="eps")
        sscale = 1.0 / INNER
        nc.vector.memset(eps_t, EPS * (WSCALE * WSCALE if PROJ_FP8 else 1.0))
        ones16 = consts.tile([128, H], F32, tag="ones16", name="ones16")
        nc.vector.memset(ones16, 1.0)
        if with_bias:
            ones_row = consts.tile([1, 128], BF16, tag="ones_row", name="ones_row")
            nc.vector.memset(ones_row, 1.0)
            b_rows = {}
            for nm, dram in (("q", bqr), ("k", bkr), ("v", bvr)):
                b_rows[nm] = consts.tile([1, INNER], BF16, tag=f"b_{nm}", name=f"b_{nm}")
                nc.sync.dma_start(out=b_rows[nm], in_=dram.ap())

        # persistent big tiles (per-partition: x 16K(fp8), w 3x16K(fp8),
        # kT 16K, qT 8K, vA 16.25K, aT 8K bf16)
        xg = big.tile([128, ND, KC], wdt, tag="xg", name="xg")
        wgt = {}
        for nm, view in (("q", wqv), ("k", wkv), ("v", wvv)):
            wgt[nm] = big.tile([128, ND, INNER], wdt, tag=f"w_{nm}", name=f"w_{nm}")
        kT = big.tile([128, NET, KC], BF16, tag="kT", name="kT")
        qT = big.tile([128, NET, BLK], BF16, tag="qT", name="qT")
        vA = [big.tile([128, VW], BF16, tag=f"vA{i}", name=f"vA{i}") for i in range(NLK)]
        vA_r = [v.rearrange("p (h c) -> p h c", c=DH + 1) for v in vA]
        aT = big.tile([128, NET, BLK], BF16, tag="aT", name="aT")
        # k-side rmsnorm scales, consumed by the attention exp()
        rsk = [consts.tile([128, 1], F32, tag=f"rsk{i}", name=f"rsk{i}") for i in range(NLK)]
        mk = [consts.tile([128, BLK], BF16, tag=f"mk{i}", name=f"mk{i}") for i in range(NLK)]

        # ---- up-front loads: the k pass starts at the HALO columns, so those
        # and wk go first; q (own cols) next; v last ----
        hd = ND // 2
        if refine:
            xlg = big.tile([128, ND, 128], FP8, tag="xlg", name="xlg")
        views = {"q": wqv, "k": wkv, "v": wvv}
        nc.sync.dma_start(out=wgt["k"][:, 0:hd, :], in_=wkv[:, 0:hd, :])
        nc.sync.dma_start(out=xg[:, 0:hd, 0:BLK], in_=x8v[:, 0:hd, 0:BLK])
        nc.sync.dma_start(out=wgt["k"][:, hd:ND, :], in_=wkv[:, hd:ND, :])
        nc.sync.dma_start(out=xg[:, hd:ND, 0:BLK], in_=x8v[:, hd:ND, 0:BLK])
        nc.sync.dma_start(out=xg[:, 0:hd, BLK:KC], in_=x8v[:, 0:hd, BLK:KC])
        nc.sync.dma_start(out=xg[:, hd:ND, BLK:KC], in_=x8v[:, hd:ND, BLK:KC])
        for nm in ("q", "v"):
            nc.sync.dma_start(out=wgt[nm][:, 0:hd, :], in_=views[nm][:, 0:hd, :])
            nc.sync.dma_start(out=wgt[nm][:, hd:ND, :], in_=views[nm][:, hd:ND, :])
        if refine:
            nc.sync.dma_start(out=xlg, in_=xlv)
        for t in range(NLK):
            nc.sync.dma_start(out=mk[t], in_=msk.ap()[t])

        # ---- projection-phase pools (released before attention) ----
        pctx = ctx.enter_context(ExitStack())
        tabs = pctx.enter_context(tc.tile_pool(name="tabs", bufs=1))
        work = pctx.enter_context(tc.tile_pool(name="work", bufs=1))
        wlp = pctx.enter_context(tc.tile_pool(name="wlp", bufs=1))
        psP = pctx.enter_context(tc.tile_pool(name="psP", bufs=1, space="PSUM"))
        psL = pctx.enter_context(tc.tile_pool(name="psL", bufs=1, space="PSUM"))
        psT = pctx.enter_context(tc.tile_pool(name="psT", bufs=1, space="PSUM"))

        def proj_ps(w, lt, bias_key):
            """x l-tile times the full weight -> ([128,1024] psum, lo-psum|None).

            lt==4 (own rows 0..127) adds an fp8 hi/lo residual pass: those rows
            feed tiny-softmax-support queries where fp8 noise doesn't average.
            """
            ref_lt = refine and lt == 4
            ps = psP.tile([128, INNER], F32, tag="pp", bufs=2, name="pp")
            pl = None
            if ref_lt:
                pl = psL.tile([128, INNER], F32, tag="pl", bufs=1, name="pl")
                wl = wlp.tile([128, ND, INNER], FP8, tag="wl", bufs=1, name="wl")
                nc.sync.dma_start(out=wl, in_=wlv[bias_key])
            for ec in range(NE2):
                es_ = slice(ec * 512, (ec + 1) * 512)
                if PROJ_FP8 and not NO_DR:
                    for dp in range(NDP):
                        nc.tensor.matmul(
                            ps[:, es_],
                            lhsT=xg[:, 2 * dp : 2 * dp + 2, lt * 128 : (lt + 1) * 128],
                            rhs=w[:, 2 * dp : 2 * dp + 2, es_],
                            start=(dp == 0),
                            stop=(dp == NDP - 1 and not with_bias),
                            perf_mode=DR,
                        )
                else:
                    for d in range(ND):
                        nc.tensor.matmul(
                            ps[:, es_],
                            lhsT=xg[:, d, lt * 128 : (lt + 1) * 128],
                            rhs=w[:, d, es_],
                            start=(d == 0),
                            stop=(d == ND - 1 and not with_bias),
                        )
                if with_bias:
                    nc.tensor.matmul(
                        ps[:, es_],
                        lhsT=ones_row,
                        rhs=b_rows[bias_key][:, es_],
                        start=False,
                        stop=True,
                    )
                if ref_lt:
                    # lo terms (both carry a 1/16 factor applied at the merge):
                    # x_lo x W_hi  +  x_hi x W_lo
                    for dp in range(NDP):
                        nc.tensor.matmul(
                            pl[:, es_],
                            lhsT=xlg[:, 2 * dp : 2 * dp + 2, :],
                            rhs=w[:, 2 * dp : 2 * dp + 2, es_],
                            start=(dp == 0),
                            stop=False,
                            perf_mode=DR,
                        )
                    for dp in range(NDP):
                        nc.tensor.matmul(
                            pl[:, es_],
                            lhsT=xg[:, 2 * dp : 2 * dp + 2, lt * 128 : (lt + 1) * 128],
                            rhs=wl[:, 2 * dp : 2 * dp + 2, es_],
                            start=False,
                            stop=(dp == NDP - 1),
                            perf_mode=DR,
                        )
            return ps, pl

        def rms_scale(praw, ss):
            """rs = 1/(WSCALE*sqrt(mean_sq_true + eps)) as a [128,1] tile."""
            rs = work.tile([128, 1], F32, tag="rs", bufs=4, name="rs")
            nc.vector.tensor_add(rs, ss[:, 0:1], ss[:, 1:2])
            nc.scalar.activation(
                rs, rs, mybir.ActivationFunctionType.Sqrt, bias=eps_t, scale=sscale
            )
            nc.vector.reciprocal(rs, rs)
            return rs

        def norm_rope_transpose(lt, ps, pl, tcos, tsin, dest, dest_w, q_side):
            """rmsnorm + rope on a projected l-tile -> transpose into dest.

            dest is a [128, NET, W] big tile; returns the k-side rs (folded
            into exp() later) or None for q (rs applied to kn here).
            """
            kdt = F32 if NO_BF16T else BF16
            tc_t = tabs.tile([128, INNER], BF16, tag="tc", bufs=2, name="tc")
            ts_t = tabs.tile([128, INNER], BF16, tag="ts", bufs=2, name="ts")
            nc.sync.dma_start(out=tc_t, in_=tcos.ap()[lt * 128 : (lt + 1) * 128])
            nc.sync.dma_start(out=ts_t, in_=tsin.ap()[lt * 128 : (lt + 1) * 128])
            praw = work.tile([128, INNER], BF16, tag="praw", bufs=3, name="praw")
            tmp = work.tile([128, INNER], kdt, tag="tmp", bufs=2, name="tmp")
            # per-512 accesses: a single engine access must not cross PSUM
            # banks, and may read at most ONE non-scalar input from PSUM
            for h0 in (0, 512):
                hs = slice(h0, h0 + 512)
                nc.scalar.copy(praw[:, hs], ps[:, hs])
                if pl is not None:
                    nc.vector.scalar_tensor_tensor(
                        out=praw[:, hs], in0=pl[:, hs], scalar=1.0 / 16.0,
                        in1=praw[:, hs], op0=MULT, op1=ADD,
                    )
            # sum of squares via ACT Square (accumulate along free dim)
            ss = work.tile([128, 2], F32, tag="ss", bufs=4, name="ss")
            for ec in range(NE2):
                hs = slice(ec * 512, (ec + 1) * 512)
                nc.scalar.activation(
                    tmp[:, hs], praw[:, hs],
                    mybir.ActivationFunctionType.Square,
                    accum_out=ss[:, ec : ec + 1],
                )
            rs = rms_scale(praw, ss)
            # rope in the [evens|odds] block layout: swap = exchange 32-blocks
            # (tmp's Square output is dead; rope overwrites it)
            kn = work.tile([128, INNER], kdt, tag="kn", bufs=2, name="kn")
            p4 = praw.rearrange("p (g t c) -> p g t c", t=2, c=32)
            t4 = tmp.rearrange("p (g t c) -> p g t c", t=2, c=32)
            s4 = ts_t.rearrange("p (g t c) -> p g t c", t=2, c=32)
            if NO_EXPSCALE and not q_side:
                q_side = True  # apply rs to kn on DVE instead of at exp()
            if q_side:
                nc.vector.scalar_tensor_tensor(
                    out=t4[:, :, 0, :], in0=p4[:, :, 1, :], scalar=rs,
                    in1=s4[:, :, 0, :], op0=MULT, op1=MULT,
                )
                nc.vector.scalar_tensor_tensor(
                    out=t4[:, :, 1, :], in0=p4[:, :, 0, :], scalar=rs,
                    in1=s4[:, :, 1, :], op0=MULT, op1=MULT,
                )
                nc.vector.scalar_tensor_tensor(
                    out=kn, in0=praw, scalar=rs, in1=tc_t, op0=MULT, op1=MULT,
                )
            else:
                nc.vector.tensor_mul(t4[:, :, 0, :], p4[:, :, 1, :], s4[:, :, 0, :])
                nc.vector.tensor_mul(t4[:, :, 1, :], p4[:, :, 0, :], s4[:, :, 1, :])
                nc.vector.tensor_mul(kn, praw, tc_t)
            nc.vector.tensor_add(kn, kn, tmp)
            # transpose [l, e] -> [e, l] into dest e-tiles (2x 4-tile groups)
            for g in range(2):
                pst = psT.tile([128, 512], kdt, tag="pst", bufs=2, name="pst")
                for i in range(4):
                    et = g * 4 + i
                    nc.tensor.transpose(
                        pst[:, i * 128 : (i + 1) * 128],
                        kn[:, et * 128 : (et + 1) * 128],
                        ident,
                    )
                nc.vector.tensor_copy(
                    dest[:, g * 4 : (g + 1) * 4, lt * 128 : (lt + 1) * 128],
                    pst.rearrange("p (i c) -> p i c", c=128),
                )
            return rs

        kTr = kT  # [128, NET, KC]
        qTr = qT  # [128, NET, BLK]

        # k pass (all 8 l-tiles) + q pass (own 4 l-tiles) share the x tile.
        for lt in range(NLK):
            kps, kpl = proj_ps(wgt["k"], lt, "k")
            rs = norm_rope_transpose(lt, kps, kpl, ktc, kts, kTr, KC, q_side=False)
            nc.vector.tensor_copy(rsk[lt], rs)
            if lt >= NLK - NLQ:
                qlt = lt - (NLK - NLQ)
                qps, qpl = proj_ps(wgt["q"], qlt + 4, "q")
                norm_rope_transpose(qlt, qps, qpl, qtc, qts, qTr, BLK, q_side=True)

        # v pass
        for lt in range(NLK):
            vps, vpl = proj_ps(wgt["v"], lt, "v")
            vct = work.tile([128, 1], F32, tag="vct", bufs=2, name="vct")
            nc.sync.dma_start(out=vct, in_=vcl.ap()[lt].rearrange("(p o) -> p o", o=1))
            nc.gpsimd.tensor_scalar_mul(vA[lt][:, DH :: DH + 1], ones16, vct)
            for ec in range(NE2):
                dst = vA_r[lt][:, ec * 8 : (ec + 1) * 8, 0:DH]
                src = vps[:, ec * 512 : (ec + 1) * 512].rearrange("p (h c) -> p h c", c=DH)
                nc.scalar.copy(dst, src)
                if vpl is not None:
                    nc.vector.scalar_tensor_tensor(
                        out=dst, scalar=1.0 / 16.0, op0=MULT, op1=ADD,
                        in0=vpl[:, ec * 512 : (ec + 1) * 512].rearrange(
                            "p (h c) -> p h c", c=DH
                        ),
                        in1=dst,
                    )

        # ---- release projection pools; open attention/output pools ----
        pctx.close()
        esp = ctx.enter_context(tc.tile_pool(name="esp", bufs=1))
        awork = ctx.enter_context(tc.tile_pool(name="awork", bufs=1))
        wp2 = ctx.enter_context(tc.tile_pool(name="wp2", bufs=1))
        outp = ctx.enter_context(tc.tile_pool(name="outp", bufs=1))
        psS = ctx.enter_context(tc.tile_pool(name="psS", bufs=1, space="PSUM"))
        psO = ctx.enter_context(tc.tile_pool(name="psO", bufs=1, space="PSUM"))
        psP2 = ctx.enter_context(tc.tile_pool(name="psP2", bufs=1, space="PSUM"))

        # ---------------- attention ----------------
        # (tile, query-half) pairs: half 0 = queries [0,256), half 1 = [256,512).
        # Valid pairs cover the causal+window+sink band; SKIP_MASK pairs are
        # fully inside the band for every core (edge cores neutralized by the
        # vcl ones-column scaling), so their exp output needs no mask multiply.
        H0_T = (0, 1, 2, 3, 4, 5)
        H1_T = (0, 2, 3, 4, 5, 6, 7)
        SKIP_MASK = {(2, 0), (3, 0), (4, 1), (5, 1)}
        # per-tile score/exp column range: the (tile, half) pairs each tile
        # participates in. Narrower bands would leave stale es regions that
        # the PV reads -- illegal under Tile's memory model.
        EXP_RANGE = {0: (0, 512), 1: (0, 256), 2: (0, 512), 3: (0, 512),
                     4: (0, 512), 5: (0, 512), 6: (256, 512), 7: (256, 512)}
        HALF = BLK // 2
        for h in range(H):
            et, r0 = h // 2, (h % 2) * 64
            es_tiles = {}
            for t in range(NLK):
                halves = [hf for hf, ts_ in ((0, H0_T), (1, H1_T)) if t in ts_]
                pss = psS.tile([128, BLK], F32, tag="pss", bufs=3, name="pss")
                es = esp.tile([128, BLK], BF16, tag="es", bufs=8, name="es")
                cs = slice(*EXP_RANGE[t])
                nc.tensor.matmul(
                    pss[:, cs],
                    lhsT=kT[r0 : r0 + 64, et, t * 128 : (t + 1) * 128],
                    rhs=qT[r0 : r0 + 64, et, cs],
                    start=True,
                    stop=True,
                    skip_group_check=True,
                )
                if NO_EXPSCALE:
                    nc.scalar.activation(
                        es[:, cs], pss[:, cs], mybir.ActivationFunctionType.Exp
                    )
                else:
                    nc.scalar.activation(
                        es[:, cs], pss[:, cs], mybir.ActivationFunctionType.Exp,
                        scale=rsk[t],
                    )
                mh = [hf for hf in halves if (t, hf) not in SKIP_MASK]
                if len(mh) == 2:
                    nc.vector.tensor_mul(es, es, mk[t])
                else:
                    for hf in mh:
                        hs = slice(hf * HALF, (hf + 1) * HALF)
                        nc.vector.tensor_mul(es[:, hs], es[:, hs], mk[t][:, hs])
                es_tiles[t] = es
            pso = psO.tile([DH + 1, BLK], F32, tag="pso", bufs=2, name="pso")
            for hf, ts_ in ((0, H0_T), (1, H1_T)):
                hs = slice(hf * HALF, (hf + 1) * HALF)
                for i, t in enumerate(ts_):
                    nc.tensor.matmul(
                        pso[:, hs],
                        lhsT=vA[t][:, h * (DH + 1) : (h + 1) * (DH + 1)],
                        rhs=es_tiles[t][:, hs],
                        start=(i == 0),
                        stop=(i == len(ts_) - 1),
                        skip_group_check=True,
                    )
            rec = awork.tile([1, BLK], F32, tag="rec", bufs=2, name="rec")
            nc.vector.reciprocal(rec, pso[DH : DH + 1, :])
            rb = awork.tile([64, BLK], F32, tag="rb", bufs=2, name="rb")
            nc.gpsimd.partition_broadcast(rb, rec)
            nc.vector.tensor_mul(aT[r0 : r0 + 64, et, :], pso[0:DH, :], rb)

        # ---------------- output projection ----------------
        for dc in range(NDC):
            wog = wp2.tile([128, NET, 512], BF16, tag="wo", bufs=2, name="wo")
            nc.sync.dma_start(out=wog, in_=wov[:, :, dc * 512 : (dc + 1) * 512])
            for lt in range(NLQ):
                po = psP2.tile([128, 512], F32, tag="po", bufs=2, name="po")
                for et in range(NET):
                    nc.tensor.matmul(
                        po,
                        lhsT=aT[:, et, lt * 128 : (lt + 1) * 128],
                        rhs=wog[:, et, :],
                        start=(et == 0),
                        stop=(et == NET - 1),
                    )
                osb = outp.tile([128, 512], F32, tag="osb", bufs=3, name="osb")
                nc.scalar.copy(osb, po)
                nc.sync.dma_start(
                    out=out.ap()[lt * 128 : (lt + 1) * 128, dc * 512 : (dc + 1) * 512],
                    in_=osb,
                )
    nc.compile()
    return nc


_NC_CACHE = {}


def get_nc(with_bias: bool = False):
    if with_bias not in _NC_CACHE:
        _NC_CACHE[with_bias] = _build(with_bias)
    return _NC_CACHE[with_bias]


def _fold_tables(cosf, sinf, w):
    """Fold rmsnorm weight w (per channel) into interleaved-rope cos/sin tables.

    rope(w*q)[2i]   = w[2i]q[2i]cos[2i]     - w[2i+1]q[2i+1]sin[2i]
    rope(w*q)[2i+1] = w[2i+1]q[2i+1]cos[2i+1] + w[2i]q[2i]sin[2i+1]
    Our kernel computes raw*tc + swap(raw)*ts, so:
      tc[:, e]    = cos[:, e]    * w[e]
      ts[:, 2i]   = -sin[:, 2i]  * w[2i+1]
      ts[:, 2i+1] =  sin[:, 2i+1]* w[2i]
    """
    tc_ = cosf * w[None, :]
    ts_ = np.empty_like(sinf)
    ts_[:, 0::2] = -sinf[:, 0::2] * w[None, 1::2]
    ts_[:, 1::2] = sinf[:, 1::2] * w[None, 0::2]
    return tc_, ts_


def _perm_old_of_new():
    """Per-head [evens|odds] channel permutation: old index for each new col."""
    p = np.empty(INNER, np.int64)
    for h in range(H):
        base = h * DH
        p[base : base + 32] = base + 2 * np.arange(32)
        p[base + 32 : base + 64] = base + 2 * np.arange(32) + 1
    return p


_PERM = _perm_old_of_new()


def _wcast(a):
    if PROJ_FP8:
        return (a * WSCALE).astype(ml_dtypes.float8_e4m3)
    return a.astype(ml_dtypes.bfloat16)


def pack_core(c, x, cosf, sinf, lls, qn_w, kn_w, Wq, Wk, Wv, Wo, bq, bk, bv, with_bias):
    b, blk = divmod(c, NBLK)
    g0 = blk * BLK

    xTc = np.zeros((D, KC), np.float32)
    xTc[:, BLK:] = x[b, g0 : g0 + BLK].T
    xTc[:, 0] = x[b, 0]
    if blk > 0:
        xTc[:, 1:BLK] = x[b, g0 - (BLK - 1) : g0].T

    # global key position per column; validity
    glob = np.empty(KC, np.int64)
    glob[0] = 0
    glob[1:BLK] = np.arange(g0 - (BLK - 1), g0)
    glob[BLK:] = np.arange(g0, g0 + BLK)
    valid = np.ones(KC, bool)
    if blk == 0:
        valid[:BLK] = False  # sink+halo columns duplicated/invalid for block 0

    # k tables indexed by column (channel-permuted)
    ktc_f, kts_f = _fold_tables(cosf, sinf, kn_w)
    ktc_c = np.zeros((KC, INNER), np.float32)
    kts_c = np.zeros((KC, INNER), np.float32)
    gv = glob[valid]
    ktc_c[valid] = ktc_f[gv]
    kts_c[valid] = kts_f[gv]

    # q tables with logit scale and 1/sqrt(dh) folded
    qtc_f, qts_f = _fold_tables(cosf, sinf, qn_w)
    qs = (lls[g0 : g0 + BLK] * (1.0 / np.sqrt(DH)))[:, None].astype(np.float32)
    qtc_c = qtc_f[g0 : g0 + BLK] * qs
    qts_c = qts_f[g0 : g0 + BLK] * qs

    # masks [NLK, 128, BLK]
    key = glob.reshape(NLK, 128)[:, :, None]  # [t, p, 1]
    q_pos = (g0 + np.arange(BLK))[None, None, :]  # [1, 1, f]
    m = (
        valid.reshape(NLK, 128)[:, :, None]
        & (key <= q_pos)
        & ((key > q_pos - WINDOW) | (key < SINK))
    ).astype(np.float32)

    P = _PERM
    im = {
        "x8": _wcast_x(xTc),
        "wq8": _wcast(np.ascontiguousarray(Wq.T)[:, P]),
        "wk8": _wcast(np.ascontiguousarray(Wk.T)[:, P]),
        "wv8": _wcast(np.ascontiguousarray(Wv.T)),
        "woT": (np.ascontiguousarray(Wo.T) * (1.0 / WSCALE if PROJ_FP8 else 1.0)).astype(
            ml_dtypes.bfloat16
        ),
        "qtc": qtc_c[:, P].astype(ml_dtypes.bfloat16),
        "qts": qts_c[:, P].astype(ml_dtypes.bfloat16),
        "ktc": ktc_c[:, P].astype(ml_dtypes.bfloat16),
        "kts": kts_c[:, P].astype(ml_dtypes.bfloat16),
        "msk": m.astype(ml_dtypes.bfloat16),
        "vcl": valid.reshape(NLK, 128).astype(np.float32),
    }
    if PROJ_FP8 and not NO_DR:
        f8 = ml_dtypes.float8_e4m3

        def lo(a):
            hi = a.astype(f8).astype(np.float32)
            return (16.0 * (a - hi)).astype(f8)

        im["x8l"] = lo(xTc[:, BLK : BLK + 128])
        im["wql"] = lo(np.ascontiguousarray(Wq.T)[:, P] * WSCALE)
        im["wkl"] = lo(np.ascontiguousarray(Wk.T)[:, P] * WSCALE)
        im["wvl"] = lo(np.ascontiguousarray(Wv.T) * WSCALE)
    if with_bias:
        sc = WSCALE if PROJ_FP8 else 1.0
        im["bqr"] = (bq[None, P] * sc).astype(ml_dtypes.bfloat16)
        im["bkr"] = (bk[None, P] * sc).astype(ml_dtypes.bfloat16)
        im["bvr"] = (bv[None, :] * sc).astype(ml_dtypes.bfloat16)
    return im


def _wcast_x(a):
    if PROJ_FP8:
        return a.astype(ml_dtypes.float8_e4m3)
    return a.astype(ml_dtypes.bfloat16)


def make_in_maps(inputs):
    f = lambda k: np.asarray(inputs[k], np.float32)
    x = f("x")
    cosf = f("cos")[0]
    sinf = f("sin")[0]
    lls = f("logit_log_scale")[0, :, 0]
    bq, bk, bv = f("bq"), f("bk"), f("bv")
    with_bias = bool(np.any(bq) or np.any(bk) or np.any(bv))
    ims = [
        pack_core(
            c, x, cosf, sinf, lls, f("qn_w"), f("kn_w"),
            f("Wq"), f("Wk"), f("Wv"), f("Wo"), bq, bk, bv, with_bias,
        )
        for c in range(NCORES)
    ]
    return ims, with_bias


last_results = None


def kernel(**inputs):
    global last_results
    ims, with_bias = make_in_maps(inputs)
    nc = get_nc(with_bias)
    res = run_bass_kernel_spmd(nc, ims, core_ids=list(range(NCORES)))
    last_results = res
    out = np.empty((B, L, D), np.float32)
    for c, om in enumerate(res.results):
        b, blk = divmod(c, NBLK)
        out[b, blk * BLK : (blk + 1) * BLK, :] = om["out"]
    out += np.asarray(inputs["bo"], np.float32)[None, None, :]
    return out
